# revision 1
# baseline (speedup 1.0000x reference)
"""Trainium2 Bass kernel for MeteoGraphSAGE (T-sharded GraphSAGE + N-sharded LSTM).

Self-contained: host-side graph preprocessing + Bass/Tile program build +
run via bass_utils.run_bass_kernel_spmd on 8 NeuronCores.

Sharding: the 8 timesteps of the GraphSAGE encoder are independent, so core t
computes the full graph encoding of timestep t (exact local BatchNorm, no
collectives).  One AllToAll then reshards to node-parallel layout for the
LSTM over time + decoder.
"""

import math
import os
import sys

import numpy as np

for _p in ("/opt/trn_rl_repo", os.path.expanduser("~/.axon_site/_ro/trn_rl_repo")):
    if os.path.isdir(_p) and _p not in sys.path:
        sys.path.insert(0, _p)

import concourse.bacc as bacc
import concourse.bass as bass
import concourse.tile as tile
from concourse import bass_utils, mybir
from concourse.masks import make_identity

F32 = mybir.dt.float32
F16 = mybir.dt.float16
I16 = mybir.dt.int16
AF = mybir.ActivationFunctionType
OP = mybir.AluOpType

BN_EPS = 1e-5


class CFG:
    def __init__(self, N, E, split=32768, T=8, F=64, H=256, O=8, BLK=64, BPC=7,
                 CPS=None):
        self.N, self.E, self.SPLIT = N, E, split
        self.T, self.F, self.H, self.O = T, F, H, O
        self.BLK, self.BPC = BLK, BPC          # segs per dst-block, blocks/chunk
        self.CHUNK = BLK * BPC                 # nodes per chunk (PSUM free dim)
        if CPS is None:                        # chunks per shard
            CPS = math.ceil(N / (T * self.CHUNK))
        self.CPS = CPS
        self.NSHARD = self.CHUNK * CPS         # nodes per core after reshard
        self.NPAD = self.NSHARD * T            # padded node count
        self.NBLK = self.NPAD // BLK
        self.NCHUNK = CPS * T                  # total chunks
        assert self.NPAD >= N


FULL = CFG(N=50000, E=1_600_000)


# ---------------------------------------------------------------- host prep
def host_prep(cfg, edge_index):
    """Sort/pad edges into (chunk, block, stream, tile) layout for dma_gather +
    selection-matrix segment sums.  Returns dict of device-ready arrays."""
    N, E, BLK, BPC = cfg.N, cfg.E, cfg.BLK, cfg.BPC
    NBLK, NCHUNK, SPLIT = cfg.NBLK, cfg.NCHUNK, cfg.SPLIT

    src = np.asarray(edge_index[0], dtype=np.int64).astype(np.int32)
    dst = np.asarray(edge_index[1], dtype=np.int64).astype(np.int32)
    counts = np.bincount(dst, minlength=N).astype(np.float32)
    recip = (1.0 / np.maximum(counts, 1.0)).astype(np.float32)
    z = (counts > 0).astype(np.float32)
    zpad = np.zeros(cfg.NPAD + 2 * cfg.CHUNK, np.float32)
    zpad[:N] = z

    stream = (src >= SPLIT).astype(np.int8)
    blk = dst // BLK
    order = np.lexsort((stream, blk))
    src_s, dst_s, st_s, blk_s = src[order], dst[order], stream[order], blk[order]

    # per (block, stream) counts
    cnt = np.zeros((NBLK, 2), np.int64)
    for s in (0, 1):
        m = st_s == s
        b, c = np.unique(blk_s[m], return_counts=True)
        cnt[b, s] = c
    TA = max(1, int(math.ceil(cnt[:, 0].max() / 128)))
    TB = max(1, int(math.ceil(cnt[:, 1].max() / 128)))

    def build(stream_id, T_s, idx_off):
        EB = T_s * 128                       # padded edges per block
        idx = np.zeros((NBLK, EB), np.int16)
        dl = np.full((NBLK, EB), -1.0, np.float32)
        rc = np.zeros((NBLK, EB), np.float32)
        m = st_s == stream_id
        sb, ss, sd = blk_s[m], src_s[m], dst_s[m]
        # edges of block b occupy a contiguous run in the sorted arrays
        starts = np.searchsorted(sb, np.arange(NBLK))
        ends = np.searchsorted(sb, np.arange(NBLK) + 1)
        for b in range(NBLK):
            n = ends[b] - starts[b]
            if n == 0:
                continue
            sl = slice(starts[b], ends[b])
            idx[b, :n] = (ss[sl] - idx_off).astype(np.int16)
            dl[b, :n] = (sd[sl] - b * BLK).astype(np.float32)
            rc[b, :n] = recip[sd[sl]].astype(np.float32)
        # chunk-major arrays (+2 dummy chunks for prefetch overrun)
        NCH2 = NCHUNK + 2
        nb2 = NCH2 * BPC
        idx2 = np.zeros((nb2, EB), np.int16)
        dl2 = np.full((nb2, EB), -1.0, np.float32)
        rc2 = np.zeros((nb2, EB), np.float32)
        idx2[:NBLK], dl2[:NBLK], rc2[:NBLK] = idx, dl, rc
        ne_c = BPC * EB                       # edges per chunk
        # dma_gather idx layout: idx i at [i%16, i//16], replicated to 128 rows
        idx_c = idx2.reshape(NCH2, ne_c)
        wrap = idx_c.reshape(NCH2, ne_c // 16, 16).transpose(0, 2, 1)  # [c,16,n16]
        idx_dev = np.tile(wrap, (1, 8, 1)).reshape(NCH2 * 128, ne_c // 16)
        # dstloc/recip: [c, 128, BPC*T_s] with (p, b*T+t) = edge (b, t*128+p)
        def dev_pt(a):
            X = a.reshape(NCH2, BPC, T_s, 128)
            return X.transpose(0, 3, 1, 2).reshape(NCH2 * 128, BPC * T_s).copy()
        return idx_dev, dev_pt(dl2), dev_pt(rc2)

    idxA, dlA, rcA = build(0, TA, 0)
    idxB, dlB, rcB = build(1, TB, SPLIT)
    return dict(TA=TA, TB=TB, idxA=idxA, dlA=dlA, rcA=rcA,
                idxB=idxB, dlB=dlB, rcB=rcB, zv=zpad[None, :].copy())


# ---------------------------------------------------------------- builder
def build_program(cfg, TA, TB, phases=9, reps=1, num_devices=None):
    N, F, H, O, T = cfg.N, cfg.F, cfg.H, cfg.O, cfg.T
    BLK, BPC, CHUNK, CPS = cfg.BLK, cfg.BPC, cfg.CHUNK, cfg.CPS
    NPAD, NSHARD, NCHUNK, SPLIT = cfg.NPAD, cfg.NSHARD, cfg.NCHUNK, cfg.SPLIT
    NPADN = float(NPAD - N)                 # number of padded (replica) nodes
    HT = H // 128                           # feature tiles (2)
    GB = 4 * H // 128                       # lstm gate blocks (8)

    nc = bacc.Bacc("TRN2", target_bir_lowering=False, debug=False,
                   enable_asserts=False,
                   num_devices=T if num_devices is None else num_devices)
    dt = lambda n, s, d: nc.dram_tensor(n, s, d, kind="ExternalInput").ap()

    xT = dt("xT", [F, NPAD], F32)
    xg = dt("xg", [N, 128], F16)
    i_idxA = dt("idxA", [(NCHUNK + 2) * 128, BPC * TA * 8], I16)
    i_idxB = dt("idxB", [(NCHUNK + 2) * 128, BPC * TB * 8], I16)
    i_dlA = dt("dlA", [(NCHUNK + 2) * 128, BPC * TA], F32)
    i_dlB = dt("dlB", [(NCHUNK + 2) * 128, BPC * TB], F32)
    i_rcA = dt("rcA", [(NCHUNK + 2) * 128, BPC * TA], F32)
    i_rcB = dt("rcB", [(NCHUNK + 2) * 128, BPC * TB], F32)
    i_zv = dt("zv", [1, NPAD + 2 * cfg.CHUNK], F32)
    i_iota = dt("iota", [128, BLK], F16)
    i_w0 = dt("w0", [F, H], F32)            # lhsT for h0
    i_wn0 = dt("wn0", [F + 1, H], F32)      # [W0 @ Wnei0 ; (Wnei0^T b0)^T]
    i_ws0 = dt("ws0", [H, H], F32)
    i_ws1 = dt("ws1", [H, H], F32)
    i_wn1 = dt("wn1", [H, H], F32)
    i_cb0 = dt("cb0", [H], F32)             # bs_self0+bs_nei0
    i_cb1 = dt("cb1", [H], F32)
    i_b0 = dt("b0v", [H], F32)
    i_g = [dt("g0", [H], F32), dt("g1", [H], F32)]
    i_bt = [dt("bt0", [H], F32), dt("bt1", [H], F32)]
    i_wih = dt("wih", [H, 4 * H], F32)      # W_ih^T
    i_whh = dt("whh", [H, 4 * H], F32)
    i_bg = dt("bg", [4 * H], F32)
    i_wdec = dt("wdec", [H, O], F32)
    i_bdec = dt("bdec", [O], F32)
    o_out = nc.dram_tensor("out", [O, NSHARD], F32, kind="ExternalOutput").ap()

    with tile.TileContext(nc) as tc:
        # ---- persistent DRAM intermediates (+2 chunks of slack: the software
        # pipeline prefetches up to 2 chunks past the end)
        NPAD2 = NPAD + 2 * CHUNK
        with tc.tile_pool(name="dram", bufs=1, space="DRAM") as dp:
            h0T = dp.tile([H, NPAD2], F32)
            h1T = dp.tile([H, NPAD2], F32)
            cmb = dp.tile([H, NPAD], F16)
            h1g = dp.tile([NPAD, H], F16)
            a2i = dp.tile([T, H, NSHARD], F16)
            a2o = dp.tile([T, H, NSHARD], F16)

        # ---- constants (SBUF resident through phase 1)
        from contextlib import ExitStack
        _phase1_ctx = ExitStack()
        cp = _phase1_ctx.enter_context(tc.tile_pool(name="consts", bufs=1))
        ident = cp.tile([128, 128], F32)
        make_identity(nc, ident[:])
        iota = cp.tile([128, BLK], F16)
        nc.sync.dma_start(out=iota[:], in_=i_iota[:, :])
        w0c = cp.tile([F, H], F32)
        nc.sync.dma_start(out=w0c[:], in_=i_w0[:, :])
        wn0 = cp.tile([F + 1, H], F32)
        nc.sync.dma_start(out=wn0[:], in_=i_wn0[:, :])
        wsk = {}
        for nm, t_in in (("ws0", i_ws0), ("ws1", i_ws1), ("wn1", i_wn1)):
            for k in range(HT):
                w = cp.tile([128, H], F32, name=f"{nm}k{k}")
                nc.sync.dma_start(out=w[:], in_=t_in[k * 128:(k + 1) * 128, :])
                wsk[nm, k] = w
        cbt, b0t, gt, btt = {}, {}, {}, {}
        for m in range(HT):
            sl = slice(m * 128, (m + 1) * 128)
            cbt[0, m] = cp.tile([128, 1], F32, name=f"cb0m{m}")
            nc.sync.dma_start(out=cbt[0, m][:], in_=i_cb0[sl, None])
            cbt[1, m] = cp.tile([128, 1], F32, name=f"cb1m{m}")
            nc.sync.dma_start(out=cbt[1, m][:], in_=i_cb1[sl, None])
            b0t[m] = cp.tile([128, 1], F32, name=f"b0m{m}")
            nc.sync.dma_start(out=b0t[m][:], in_=i_b0[sl, None])
            for li in range(2):
                gt[li, m] = cp.tile([128, 1], F32, name=f"g{li}m{m}")
                nc.sync.dma_start(out=gt[li, m][:], in_=i_g[li][sl, None])
                btt[li, m] = cp.tile([128, 1], F32, name=f"bt{li}m{m}")
                nc.sync.dma_start(out=btt[li, m][:], in_=i_bt[li][sl, None])

        # statistics accumulators + BN affine + pad-replica state
        sacc = _phase1_ctx.enter_context(tc.tile_pool(name="sacc", bufs=1))
        ssum = {(li, m): sacc.tile([128, 1], F32, name=f"ssum{li}{m}")
                for li in range(2) for m in range(HT)}
        ssq = {(li, m): sacc.tile([128, 1], F32, name=f"ssq{li}{m}")
               for li in range(2) for m in range(HT)}
        bnsc = {(li, m): sacc.tile([128, 1], F32, name=f"bnsc{li}{m}")
                for li in range(2) for m in range(HT)}
        bnbi = {(li, m): sacc.tile([128, 1], F32, name=f"bnbi{li}{m}")
                for li in range(2) for m in range(HT)}
        padh = {(li, m): sacc.tile([128, 1], F32, name=f"padh{li}{m}")
                for li in range(3) for m in range(HT)}
        epsT = sacc.tile([128, 1], F32, name="epsT")
        nc.vector.memset(epsT[:], BN_EPS)

        def _init_stats():
            for li in range(2):
                for m in range(HT):
                    nc.vector.memset(ssum[li, m][:], 0.0)
                    nc.vector.memset(ssq[li, m][:], 0.0)
            for m in range(HT):
                nc.vector.tensor_copy(out=padh[0, m][:], in_=b0t[m][:])

        # zero-fill the +2-chunk prefetch slack of h0T/h1T
        with tc.tile_pool(name="zf", bufs=1) as zf:
            zt = zf.tile([128, 2 * CHUNK], F32)
            nc.vector.memset(zt[:], 0.0)
            for m in range(HT):
                msl = slice(m * 128, (m + 1) * 128)
                nc.sync.dma_start(out=h0T[msl, NPAD:NPAD2], in_=zt[:])
                nc.sync.dma_start(out=h1T[msl, NPAD:NPAD2], in_=zt[:])

        # ================= phase 0: h0^T = W0^T x^T + b0 (python unrolled)
        with tc.tile_pool(name="p0", bufs=3) as p0, \
                tc.tile_pool(name="p0ps", bufs=4, space="PSUM") as p0ps:
            for c in range(NCHUNK):
                csl = slice(c * CHUNK, (c + 1) * CHUNK)
                xc = p0.tile([F, CHUNK], F32)
                nc.sync.dma_start(out=xc[:], in_=xT[:, csl])
                for m in range(HT):
                    ps = p0ps.tile([128, CHUNK], F32, tag="ps")
                    nc.tensor.matmul(ps[:], lhsT=w0c[:, m * 128:(m + 1) * 128],
                                     rhs=xc[:], start=True, stop=True)
                    ho = p0.tile([128, CHUNK], F32, tag="ho")
                    nc.vector.tensor_scalar(ho[:], ps[:], b0t[m][:], 0.0, OP.add, OP.add)
                    nc.sync.dma_start(out=h0T[m * 128:(m + 1) * 128, csl],
                                      in_=ho[:])

        # ================= pass 1 of a layer (For_i over chunk pairs)
        def pass1(li):
            # layer 0 gathers x rows (padded to 128 f16); layer 1 gathers h1
            ELEM = 128 if li == 0 else H
            srcA = xg[:, :] if li == 0 else h1g[:, :]
            srcB = xg[SPLIT:, :] if li == 0 else h1g[SPLIT:, :]
            hprev = h0T if li == 0 else h1T
            nA, nB = BPC * TA * 128, BPC * TB * 128
            KA, KB = nA // 16, nB // 16

            with tc.tile_pool(name=f"g{li}", bufs=1) as gp, \
                    tc.tile_pool(name=f"w{li}", bufs=2) as wp, \
                    tc.tile_pool(name=f"s{li}", bufs=6) as sp, \
                    tc.tile_pool(name=f"ps{li}", bufs=2, space="PSUM") as pp, \
                    tc.tile_pool(name=f"cps{li}", bufs=2, space="PSUM") as cpp:
                # parity-static slots (manual double buffering across For_i)
                slots = []
                for par in range(2):
                    s = dict(
                        gA=gp.tile([128, BPC * TA, ELEM], F16, name=f"gA{par}"),
                        gB=gp.tile([128, BPC * TB, ELEM], F16, name=f"gB{par}"),
                        ixA=gp.tile([128, KA], I16, name=f"ixA{par}"),
                        ixB=gp.tile([128, KB], I16, name=f"ixB{par}"),
                        dlA=gp.tile([128, BPC * TA], F32, name=f"dlA{par}"),
                        dlB=gp.tile([128, BPC * TB], F32, name=f"dlB{par}"),
                        rcA=gp.tile([128, BPC * TA], F32, name=f"rcA{par}"),
                        rcB=gp.tile([128, BPC * TB], F32, name=f"rcB{par}"),
                        hp=[gp.tile([128, CHUNK], F32, name=f"hp{par}{m}")
                            for m in range(HT)],
                        zr=gp.tile([1, CHUNK], F32, name=f"zr{par}") if li == 0
                        else None,
                    )
                    slots.append(s)

                def fetch(s, coff):
                    # coff = chunk index expression (python int or reg expr)
                    r128 = coff * 128
                    nc.sync.dma_start(out=s["ixA"][:],
                                      in_=i_idxA[bass.ds(r128, 128), :])
                    nc.sync.dma_start(out=s["ixB"][:],
                                      in_=i_idxB[bass.ds(r128, 128), :])
                    nc.sync.dma_start(out=s["dlA"][:],
                                      in_=i_dlA[bass.ds(r128, 128), :])
                    nc.sync.dma_start(out=s["dlB"][:],
                                      in_=i_dlB[bass.ds(r128, 128), :])
                    nc.sync.dma_start(out=s["rcA"][:],
                                      in_=i_rcA[bass.ds(r128, 128), :])
                    nc.sync.dma_start(out=s["rcB"][:],
                                      in_=i_rcB[bass.ds(r128, 128), :])
                    nc.gpsimd.dma_gather(s["gA"][:], srcA, s["ixA"][:], nA, nA,
                                         ELEM, single_packet=False)
                    nc.gpsimd.dma_gather(s["gB"][:], srcB, s["ixB"][:], nB, nB,
                                         ELEM, single_packet=False)

                def fetch_h(s, coff):
                    cs = coff * CHUNK
                    for m in range(HT):
                        nc.sync.dma_start(
                            out=s["hp"][m][:],
                            in_=hprev[m * 128:(m + 1) * 128, bass.ds(cs, CHUNK)])
                    if li == 0:
                        nc.sync.dma_start(out=s["zr"][:],
                                          in_=i_zv[0:1, bass.ds(cs, CHUNK)])

                def compute(s, coff):
                    cs = coff * CHUNK
                    if li == 0:
                        agg = wp.tile([F + 1, CHUNK], F32, tag="agg0")
                        nc.vector.tensor_copy(out=agg[F:F + 1, :], in_=s["zr"][:])
                        aggk = [agg]
                    else:
                        aggk = [wp.tile([128, CHUNK], F32, tag=f"agg{k}",
                                            name=f"agg{k}t")
                                for k in range(HT)]
                    for b in range(BPC):
                        if li == 0:
                            ps = [pp.tile([128, BLK], F32, tag="aps0", name=f"aps0b{b}")]
                        else:
                            ps = [pp.tile([128, BLK], F32, tag=f"aps{k}",
                                           name=f"aps{k}b{b}")
                                  for k in range(HT)]
                        streams = ((s["gA"], s["dlA"], s["rcA"], TA),
                                   (s["gB"], s["dlB"], s["rcB"], TB))
                        for si, (g, dl, rc, TS) in enumerate(streams):
                            for t in range(TS):
                                j = b * TS + t
                                sel = sp.tile([128, BLK], F16, tag="sel")
                                nc.vector.tensor_scalar(
                                    sel[:], iota[:], dl[:, j:j + 1],
                                    rc[:, j:j + 1], OP.is_equal, OP.mult)
                                for k in range(len(ps)):
                                    nc.tensor.matmul(
                                        ps[k][:],
                                        lhsT=g[:, j, k * 128:(k + 1) * 128],
                                        rhs=sel[:],
                                        start=(si == 0 and t == 0),
                                        stop=(si == 1 and t == TS - 1))
                        bs = slice(b * BLK, (b + 1) * BLK)
                        if li == 0:
                            nc.vector.tensor_copy(out=aggk[0][0:F, bs],
                                                  in_=ps[0][0:F, :])
                        else:
                            for k in range(HT):
                                nc.vector.tensor_copy(out=aggk[k][:, bs],
                                                      in_=ps[k][:, :])
                    # comb^T = Wself^T h^T + Wnei^T agg^T (+bias) ; stats
                    for m in range(HT):
                        msl = slice(m * 128, (m + 1) * 128)
                        cps = cpp.tile([128, CHUNK], F32, tag="cps")
                        nm = "ws0" if li == 0 else "ws1"
                        nc.tensor.matmul(cps[:], lhsT=wsk[nm, 0][:, msl],
                                         rhs=s["hp"][0][:], start=True,
                                         stop=False)
                        nc.tensor.matmul(cps[:], lhsT=wsk[nm, 1][:, msl],
                                         rhs=s["hp"][1][:], start=False,
                                         stop=False)
                        if li == 0:
                            nc.tensor.matmul(cps[:], lhsT=wn0[:, msl],
                                             rhs=aggk[0][:], start=False,
                                             stop=True)
                        else:
                            nc.tensor.matmul(cps[:], lhsT=wsk["wn1", 0][:, msl],
                                             rhs=aggk[0][:], start=False,
                                             stop=False)
                            nc.tensor.matmul(cps[:], lhsT=wsk["wn1", 1][:, msl],
                                             rhs=aggk[1][:], start=False,
                                             stop=True)
                        c16 = wp.tile([128, CHUNK], F16, tag="c16")
                        tsum = sp.tile([128, 1], F32, tag="tsum")
                        nc.vector.tensor_scalar(c16[:], cps[:], cbt[li, m][:],
                                                0.0, OP.add, OP.add,
                                                accum_out=tsum[:])
                        nc.vector.tensor_add(ssum[li, m][:], ssum[li, m][:],
                                             tsum[:])
                        sq = wp.tile([128, CHUNK], F32, tag="sq")
                        tsq = sp.tile([128, 1], F32, tag="tsq")
                        nc.scalar.activation(sq[:], cps[:], AF.Square,
                                             bias=cbt[li, m][:], scale=1.0,
                                             accum_out=tsq[:])
                        nc.vector.tensor_add(ssq[li, m][:], ssq[li, m][:],
                                             tsq[:])
                        nc.sync.dma_start(out=cmb[msl, bass.ds(cs, CHUNK)],
                                          in_=c16[:])

                # prologue: fetch chunk 0 into even slot
                fetch(slots[0], 0)
                fetch_h(slots[0], 0)
                with tc.For_i(0, NCHUNK // 2, 1) as it:
                    c0 = it * 2
                    fetch(slots[1], c0 + 1)
                    fetch_h(slots[1], c0 + 1)
                    compute(slots[0], c0)
                    fetch(slots[0], c0 + 2)
                    fetch_h(slots[0], c0 + 2)
                    compute(slots[1], c0 + 1)

        # ================= between-pass epilogue: BN affine from stats
        def bn_finalize(li):
            with tc.tile_pool(name=f"bn{li}", bufs=2) as bp, \
                    tc.tile_pool(name=f"bnps{li}", bufs=2, space="PSUM") as bpp:
                nm = "ws0" if li == 0 else "ws1"
                for m in range(HT):
                    msl = slice(m * 128, (m + 1) * 128)
                    # pad-replica comb value
                    pp_ = bpp.tile([128, 1], F32, tag="pp")
                    nc.tensor.matmul(pp_[:], lhsT=wsk[nm, 0][:, msl],
                                     rhs=padh[li, 0][:], start=True, stop=False)
                    nc.tensor.matmul(pp_[:], lhsT=wsk[nm, 1][:, msl],
                                     rhs=padh[li, 1][:], start=False, stop=True)
                    padc = bp.tile([128, 1], F32, tag="padc")
                    nc.vector.tensor_scalar(padc[:], pp_[:], cbt[li, m][:],
                                            0.0, OP.add, OP.add)
                    # remove pad contribution from stats
                    tmp = bp.tile([128, 1], F32, tag="tmp")
                    nc.vector.tensor_scalar(tmp[:], padc[:], -NPADN, 0.0,
                                            OP.mult, OP.add)
                    nc.vector.tensor_add(ssum[li, m][:], ssum[li, m][:], tmp[:])
                    sq2 = bp.tile([128, 1], F32, tag="sq2")
                    nc.scalar.activation(sq2[:], padc[:], AF.Square)
                    nc.vector.tensor_scalar(sq2[:], sq2[:], -NPADN, 0.0,
                                            OP.mult, OP.add)
                    nc.vector.tensor_add(ssq[li, m][:], ssq[li, m][:], sq2[:])
                    # mu, var, rstd, scale, bias
                    mu = bp.tile([128, 1], F32, tag="mu")
                    nc.vector.tensor_scalar(mu[:], ssum[li, m][:], 1.0 / N,
                                            0.0, OP.mult, OP.add)
                    var = bp.tile([128, 1], F32, tag="var")
                    nc.vector.tensor_scalar(var[:], ssq[li, m][:], 1.0 / N,
                                            0.0, OP.mult, OP.add)
                    musq = bp.tile([128, 1], F32, tag="musq")
                    nc.vector.tensor_tensor(out=musq[:], in0=mu[:], in1=mu[:],
                                            op=OP.mult)
                    nc.vector.tensor_tensor(out=var[:], in0=var[:], in1=musq[:],
                                            op=OP.subtract)
                    std = bp.tile([128, 1], F32, tag="std")
                    nc.scalar.activation(std[:], var[:], AF.Sqrt, bias=epsT[:])
                    rstd = bp.tile([128, 1], F32, tag="rstd")
                    nc.vector.reciprocal(rstd[:], std[:])
                    nc.vector.tensor_tensor(out=bnsc[li, m][:], in0=gt[li, m][:],
                                            in1=rstd[:], op=OP.mult)
                    mt = bp.tile([128, 1], F32, tag="mt")
                    nc.vector.tensor_tensor(out=mt[:], in0=mu[:],
                                            in1=bnsc[li, m][:], op=OP.mult)
                    nc.vector.tensor_tensor(out=bnbi[li, m][:], in0=btt[li, m][:],
                                            in1=mt[:], op=OP.subtract)
                    # pad-replica h update
                    pr = bp.tile([128, 1], F32, tag="pr")
                    nc.scalar.activation(pr[:], padc[:], AF.Relu,
                                         bias=bnbi[li, m][:],
                                         scale=bnsc[li, m][:])
                    nc.vector.tensor_add(padh[li + 1, m][:], padh[li, m][:],
                                         pr[:])

        # ================= pass 2 of a layer (unrolled)
        def pass2(li):
            hprev = h0T if li == 0 else h1T
            with tc.tile_pool(name=f"q{li}", bufs=4) as qp, \
                    tc.tile_pool(name=f"qps{li}", bufs=4, space="PSUM") as qpp:
                for c in range(NCHUNK):
                    csl = slice(c * CHUNK, (c + 1) * CHUNK)
                    hn = []
                    for m in range(HT):
                        msl = slice(m * 128, (m + 1) * 128)
                        c16 = qp.tile([128, CHUNK], F16, tag="c16")
                        nc.sync.dma_start(out=c16[:], in_=cmb[msl, csl])
                        hp = qp.tile([128, CHUNK], F32, tag="hp")
                        nc.sync.dma_start(out=hp[:], in_=hprev[msl, csl])
                        rl = qp.tile([128, CHUNK], F32, tag="rl")
                        nc.scalar.activation(rl[:], c16[:], AF.Relu,
                                             bias=bnbi[li, m][:],
                                             scale=bnsc[li, m][:])
                        h_ = qp.tile([128, CHUNK], F32, tag="hn")
                        nc.vector.tensor_add(h_[:], hp[:], rl[:])
                        hn.append(h_)
                        if li == 0:
                            nc.sync.dma_start(out=h1T[msl, csl], in_=h_[:])
                        else:
                            sh, cc = divmod(c, CPS)
                            # f32 -> f16 cast during DMA requires SWDGE
                            nc.gpsimd.dma_start(
                                out=a2i[sh, msl, cc * CHUNK:(cc + 1) * CHUNK],
                                in_=h_[:])
                    if li == 0:
                        # node-major f16 copy for layer-2 gather (PE transpose)
                        for nb in range(BPC * BLK // 128 + (1 if CHUNK % 128 else 0)):
                            w = min(128, CHUNK - nb * 128)
                            n16 = qp.tile([128, H], F16, tag="n16")
                            for m in range(HT):
                                tp = qpp.tile([128, 128], F32, tag="tp")
                                nc.tensor.transpose(
                                    tp[:w, :], hn[m][:, nb * 128:nb * 128 + w],
                                    ident[:])
                                nc.vector.tensor_copy(
                                    out=n16[:w, m * 128:(m + 1) * 128],
                                    in_=tp[:w, :])
                            nc.sync.dma_start(
                                out=h1g[c * CHUNK + nb * 128:
                                        c * CHUNK + nb * 128 + w, :],
                                in_=n16[:w, :])

        for _rep in range(reps):
            _init_stats()
            if phases >= 2:
                pass1(0)
                bn_finalize(0)
            if phases >= 3:
                pass2(0)
            if phases >= 4:
                pass1(1)
                bn_finalize(1)
            if phases >= 5:
                pass2(1)
            if phases >= 6:
                nc.gpsimd.collective_compute(
                    "AllToAll", OP.bypass, replica_groups=[list(range(T))],
                    ins=[a2i[:]], outs=[a2o[:]])

        _phase1_ctx.close()

        if phases < 7:
            with tc.tile_pool(name="zout", bufs=1) as zo:
                zt2 = zo.tile([O, CHUNK], F32)
                nc.vector.memset(zt2[:], 0.0)
                for c in range(CPS):
                    nc.sync.dma_start(out=o_out[:, c * CHUNK:(c + 1) * CHUNK],
                                      in_=zt2[:])

        # ================= LSTM over time (node-parallel) + decoder
        NHALF = NSHARD // 2
        CH = NHALF // CHUNK                    # chunks per half
        def _lstm_phase():
            with tc.tile_pool(name="lw", bufs=1) as lw, \
                    tc.tile_pool(name="lst", bufs=1) as ls, \
                    tc.tile_pool(name="lwk", bufs=3) as lk, \
                    tc.tile_pool(name="lps", bufs=4, space="PSUM") as lp:
                wih = [lw.tile([128, 4 * H], F16, name=f"wih{k}") for k in range(HT)]
                whh = [lw.tile([128, 4 * H], F16, name=f"whh{k}") for k in range(HT)]
                for k in range(HT):
                    nc.gpsimd.dma_start(out=wih[k][:],
                                        in_=i_wih[k * 128:(k + 1) * 128, :])
                    nc.gpsimd.dma_start(out=whh[k][:],
                                        in_=i_whh[k * 128:(k + 1) * 128, :])
                bgt = [lw.tile([128, 1], F32, name=f"bg{g}") for g in range(GB)]
                for g in range(GB):
                    nc.sync.dma_start(out=bgt[g][:],
                                      in_=i_bg[g * 128:(g + 1) * 128, None])
                bdt = lw.tile([O, 1], F32)
                nc.sync.dma_start(out=bdt[:], in_=i_bdec[:, None])
                wdt = [lw.tile([128, O], F16, name=f"wd{k}") for k in range(HT)]
                for k in range(HT):
                    nc.gpsimd.dma_start(out=wdt[k][:],
                                        in_=i_wdec[k * 128:(k + 1) * 128, :])

                cst = [ls.tile([128, NSHARD], F32, name=f"c{m}") for m in range(HT)]
                hst = [ls.tile([128, NSHARD], F16, name=f"h{m}") for m in range(HT)]
                gst = [ls.tile([128, NHALF], F16, name=f"gs{g}") for g in range(GB)]
                eh = [ls.tile([128, NHALF], F16, name=f"e{k}") for k in range(HT)]
                for m in range(HT):
                    nc.vector.memset(cst[m][:], 0.0)
                    nc.vector.memset(hst[m][:], 0.0)

                for step in range(T):
                    for half in range(2):
                        hoff = half * NHALF
                        for k in range(HT):
                            nc.sync.dma_start(
                                out=eh[k][:],
                                in_=a2o[step, k * 128:(k + 1) * 128,
                                        hoff:hoff + NHALF])
                        for g in range(GB):
                            gsl = slice(g * 128, (g + 1) * 128)
                            fn = AF.Tanh if g in (4, 5) else AF.Sigmoid
                            for ch in range(CH):
                                s0, s1 = ch * CHUNK, (ch + 1) * CHUNK
                                ps = lp.tile([128, CHUNK], F32, tag="gps")
                                nc.tensor.matmul(ps[:], lhsT=wih[0][:, gsl],
                                                 rhs=eh[0][:, s0:s1], start=True,
                                                 stop=False)
                                nc.tensor.matmul(ps[:], lhsT=wih[1][:, gsl],
                                                 rhs=eh[1][:, s0:s1], start=False,
                                                 stop=False)
                                nc.tensor.matmul(
                                    ps[:], lhsT=whh[0][:, gsl],
                                    rhs=hst[0][:, hoff + s0:hoff + s1],
                                    start=False, stop=False)
                                nc.tensor.matmul(
                                    ps[:], lhsT=whh[1][:, gsl],
                                    rhs=hst[1][:, hoff + s0:hoff + s1],
                                    start=False, stop=True)
                                nc.scalar.activation(gst[g][:, s0:s1], ps[:], fn,
                                                     bias=bgt[g][:])
                        for ch in range(CH):
                            s0, s1 = ch * CHUNK, (ch + 1) * CHUNK
                            for m in range(HT):
                                csl_ = cst[m][:, hoff + s0:hoff + s1]
                                t1 = lk.tile([128, CHUNK], F32, tag="t1")
                                nc.vector.tensor_tensor(
                                    out=t1[:], in0=gst[2 + m][:, s0:s1],
                                    in1=csl_, op=OP.mult)
                                t2 = lk.tile([128, CHUNK], F32, tag="t2")
                                nc.vector.tensor_tensor(
                                    out=t2[:], in0=gst[0 + m][:, s0:s1],
                                    in1=gst[4 + m][:, s0:s1], op=OP.mult)
                                nc.vector.tensor_tensor(out=csl_, in0=t1[:],
                                                        in1=t2[:], op=OP.add)
                                t3 = lk.tile([128, CHUNK], F32, tag="t3")
                                nc.scalar.activation(t3[:], csl_, AF.Tanh)
                                nc.vector.tensor_tensor(
                                    out=hst[m][:, hoff + s0:hoff + s1],
                                    in0=gst[6 + m][:, s0:s1], in1=t3[:],
                                    op=OP.mult)
                # decoder
                for c in range(CPS):
                    s0, s1 = c * CHUNK, (c + 1) * CHUNK
                    ps = lp.tile([O, CHUNK], F32, tag="dps")
                    nc.tensor.matmul(ps[:], lhsT=wdt[0][:], rhs=hst[0][:, s0:s1],
                                     start=True, stop=False)
                    nc.tensor.matmul(ps[:], lhsT=wdt[1][:], rhs=hst[1][:, s0:s1],
                                     start=False, stop=True)
                    ob = lk.tile([O, CHUNK], F32, tag="ob")
                    nc.vector.tensor_scalar(ob[:], ps[:], bdt[:], 0.0, OP.add, OP.add)
                    nc.sync.dma_start(out=o_out[:, s0:s1], in_=ob[:])


        if phases >= 7:
            _lstm_phase()

    nc.compile()
    return nc


# ---------------------------------------------------------------- driver
def _make_in_maps(cfg, prep, x, W0, b0, Ws_self, bs_self, Ws_nei, bs_nei,
                  gamma, beta, W_ih, W_hh, b_ih, b_hh, W_dec, b_dec):
    N, F, NPAD, T = cfg.N, cfg.F, cfg.NPAD, cfg.T
    W0 = np.asarray(W0, np.float32)
    b0 = np.asarray(b0, np.float32)
    Ws_nei = np.asarray(Ws_nei, np.float32)
    wn0 = np.concatenate([W0 @ Ws_nei[0],
                          (Ws_nei[0].T @ b0)[None, :]], axis=0)
    common = dict(
        idxA=prep["idxA"], idxB=prep["idxB"], dlA=prep["dlA"], dlB=prep["dlB"],
        rcA=prep["rcA"], rcB=prep["rcB"], zv=prep["zv"],
        iota=np.broadcast_to(np.arange(cfg.BLK, dtype=np.float16),
                             (128, cfg.BLK)).copy(),
        w0=W0, wn0=wn0.astype(np.float32),
        ws0=np.asarray(Ws_self[0], np.float32),
        ws1=np.asarray(Ws_self[1], np.float32),
        wn1=Ws_nei[1],
        cb0=(np.asarray(bs_self[0]) + np.asarray(bs_nei[0])).astype(np.float32),
        cb1=(np.asarray(bs_self[1]) + np.asarray(bs_nei[1])).astype(np.float32),
        b0v=b0, g0=np.asarray(gamma[0], np.float32),
        g1=np.asarray(gamma[1], np.float32),
        bt0=np.asarray(beta[0], np.float32),
        bt1=np.asarray(beta[1], np.float32),
        wih=np.ascontiguousarray(np.asarray(W_ih, np.float32).T),
        whh=np.ascontiguousarray(np.asarray(W_hh, np.float32).T),
        bg=(np.asarray(b_ih) + np.asarray(b_hh)).astype(np.float32),
        wdec=np.asarray(W_dec, np.float32),
        bdec=np.asarray(b_dec, np.float32),
    )
    x = np.asarray(x, np.float32)
    in_maps = []
    for t in range(T):
        xTt = np.zeros((F, NPAD), np.float32)
        xTt[:, :N] = x[t].T
        xgt = np.zeros((N, 128), np.float16)
        xgt[:, :F] = x[t].astype(np.float16)
        in_maps.append(dict(common, xT=xTt, xg=xgt))
    return in_maps


def run(cfg, inputs, trace=False):
    prep = host_prep(cfg, inputs["edge_index"])
    nc = build_program(cfg, prep["TA"], prep["TB"])
    in_maps = _make_in_maps(cfg, prep, **{k: v for k, v in inputs.items()
                                          if k != "edge_index"})
    res = bass_utils.run_bass_kernel_spmd(
        nc, in_maps, core_ids=list(range(cfg.T)), trace=trace)
    outs = [res.results[c]["out"] for c in range(cfg.T)]
    full = np.concatenate(outs, axis=1)         # [O, NPAD]
    return np.ascontiguousarray(full.T[:cfg.N]), res


def kernel(**inputs):
    cfg = FULL
    out, _ = run(cfg, inputs, trace=bool(os.environ.get("BASS_TRACE")))
    return out.astype(np.float32)

